# revision 20
# baseline (speedup 1.0000x reference)
"""Axial attention (no softmax) on 8 TRN2 NeuronCores.

Problem: x (8, 64, 64, 1024) fp32; two self-attentions (16 heads, no
softmax, scale d**-0.5) along the H axis (w_qkv0/w_out0) and the W axis
(w_qkv1/w_out1); output is their sum.

Sharding: data-parallel over batch B=8 -> one batch slab per core,
weights replicated. Each core computes both axial passes for its slab;
no collectives.

Per-core kernel structure (all matmuls bf16, fp32 PSUM accumulate):
  tokens t = h*64 + w (h-major), NT = 4096 per slab.
  For each pass (H-axis then W-axis), in chunks of 8 sequences
  (CH = 512 tokens, chunk token order is sequence-major):
    1. DMA natural x tiles [128 tok, 1024], PE-transpose to
       xT [128 d, 512 tok] tiles (8 k-tiles per chunk).
    2. qkT[m] = (Wqk[:, m-block]).T @ xT  -> [128 qk-dim, 512 tok]
       (16 m-tiles, 8 k accumulation steps each; q scaled by 1/32).
    3. v[tb] = x @ Wv -> [128 tok, 1024] natural layout (4 tok-blocks).
    4. Per (head-pair j, seq-pair sp): 4-way 64x64 tile_position packs:
       A^T = kT.T @ qT   (4 matmuls into one PSUM tile)
       O^T = v.T  @ A^T  (4 matmuls into one PSUM tile)
       assembling OT[j] [128 d, 512 tok].
    5. y = OT.T @ Wout -> [128 tok, 512] fp32; pass H writes out
       directly, pass W gpsimd-DMA-accumulates (out = oh + ow).
"""

import numpy as np
import ml_dtypes
from contextlib import ExitStack

from concourse.bass_utils import run_bass_kernel_spmd
from concourse import bacc, mybir, tile
from concourse.masks import make_identity

BF16 = mybir.dt.bfloat16
F32 = mybir.dt.float32

B = 8
D = 1024
NT = 4096          # tokens per core (64*64)
CH = 512           # chunk tokens (8 sequences of 64)
NCHUNK = NT // CH  # 8
KB = D // 128      # 8 contraction blocks
SCALE = 1.0 / 32.0  # 1024 ** -0.5

_BUILD_CACHE = {}
STAGE_MAP = {}


class _TensorProxy:
    """Records which pipeline stage emitted each PE instruction (for
    trace attribution in the perf harness)."""

    def __init__(self, te):
        self._te = te
        self.stage = "?"

    def matmul(self, *a, **kw):
        r = self._te.matmul(*a, **kw)
        STAGE_MAP[r.ins.name] = self.stage
        return r

    def transpose(self, *a, **kw):
        r = self._te.transpose(*a, **kw)
        STAGE_MAP[r.ins.name] = self.stage
        return r


def build(n_chunks=NCHUNK, passes=(0, 1)):
    key = (n_chunks, tuple(passes))
    if key in _BUILD_CACHE:
        return _BUILD_CACHE[key]

    nc = bacc.Bacc("TRN2", target_bir_lowering=False, debug=False)
    x = nc.dram_tensor("x", [NT, D], BF16, kind="ExternalInput")
    wqk = [nc.dram_tensor(f"wqk{p}", [D, 2 * D], BF16, kind="ExternalInput")
           for p in range(2)]
    wv = [nc.dram_tensor(f"wv{p}", [D, D], BF16, kind="ExternalInput")
          for p in range(2)]
    wo = [nc.dram_tensor(f"wo{p}", [D, D], BF16, kind="ExternalInput")
          for p in range(2)]
    out = nc.dram_tensor("out", [NT, D], F32, kind="ExternalOutput")

    xg = x.rearrange("(h w) d -> w h d", w=64)    # pass-H gather view
    og = out.rearrange("(h w) d -> w h d", w=64)  # pass-H scatter view

    with tile.TileContext(nc) as tc, ExitStack() as ctx:
        def pool(name, bufs, space="SBUF"):
            return ctx.enter_context(
                tc.tile_pool(name=name, bufs=bufs, space=space))

        p_id = pool("ident", 1)
        p_wqk = pool("wqk", 16)
        p_wv = pool("wv", 8)
        p_wo = pool("wo", 8)
        p_xn = pool("xn", 8)
        p_xt = pool("xt", 16)
        p_qkt = pool("qkt", 16)
        p_v = pool("v", 8)
        p_sa = pool("sa", 8)
        p_ot = pool("ot", 16)
        p_y = pool("y", 6)
        # PSUM budget: 8 banks total (each tile is padded to one bank).
        # Row-tiled 64x64 matmul packs need the two row tiles' outputs in
        # DIFFERENT banks (concurrent row tiles may not share a bank).
        ps_big = pool("psb", 3, "PSUM")    # [128, 512] f32 qkv/y groups
        ps_att = pool("psatt", 5, "PSUM")  # transpose + A^T/O^T halves

        te = _TensorProxy(nc.tensor)
        ident = p_id.tile([128, 128], BF16, name="ident")
        make_identity(nc, ident)

        # PE warm-up: ~5us of dummy matmuls while the first DMAs land,
        # so the HAM clock gate reaches 8/8 before real work starts.
        te.stage = "warm"
        warm_ps = ps_big.tile([128, 128], F32, tag="big", name="warm_ps")
        for _ in range(40):
            te.matmul(warm_ps[:], lhsT=ident[:], rhs=ident[:],
                      start=True, stop=True)

        for p in passes:
            if p == passes[0]:
                # prefetch chunk-0 x tiles ahead of the weight stream
                pre_xns = []
                for tb in range(4):
                    xn = p_xn.tile([128, D], BF16, tag="xn", name=f"xn_pre_{p}_{tb}")
                    engs = (nc.sync, nc.scalar, nc.gpsimd)
                    eng = engs[tb % 3]
                    if p == 1:
                        eng.dma_start(xn[:], x[tb * 128:(tb + 1) * 128, :])
                    else:
                        eng.dma_start(xn[:], xg[tb * 2:tb * 2 + 2, :, :])
                    pre_xns.append(xn)
            else:
                pre_xns = None
            wqk_t = []
            for k in range(KB):
                t = p_wqk.tile([128, 2 * D], BF16, tag="wqk", name=f"wqk_{p}_{k}")
                nc.sync.dma_start(t[:], wqk[p][k * 128:(k + 1) * 128, :])
                wqk_t.append(t)
            wv_t = []
            for k in range(KB):
                t = p_wv.tile([128, D], BF16, tag="wv", name=f"wv_{p}_{k}")
                nc.scalar.dma_start(t[:], wv[p][k * 128:(k + 1) * 128, :])
                wv_t.append(t)
            wo_t = []
            for k in range(KB):
                t = p_wo.tile([128, D], BF16, tag="wo", name=f"wo_{p}_{k}")
                nc.scalar.dma_start(t[:], wo[p][k * 128:(k + 1) * 128, :])
                wo_t.append(t)

            for c in range(n_chunks):
                # 1. load natural x tiles, PE-transpose into xT k-tiles.
                # All 4 transposes of one k-block go into one [128, 512]
                # PSUM tile (one bank, one copy out).
                xt = [p_xt.tile([128, CH], BF16, tag="xt", name=f"xt_{p}_{c}_{i}") for i in range(KB)]
                if c == 0 and pre_xns is not None:
                    xns = pre_xns
                else:
                    xns = []
                    for tb in range(4):
                        xn = p_xn.tile([128, D], BF16, tag="xn", name=f"xn_{p}_{c}_{tb}")
                        engs = (nc.sync, nc.scalar, nc.gpsimd)
                        eng = engs[(c * 4 + tb) % 3]
                        if p == 1:
                            t0 = c * CH + tb * 128
                            eng.dma_start(xn[:], x[t0:t0 + 128, :])
                        else:
                            w0 = c * 8 + tb * 2
                            eng.dma_start(xn[:], xg[w0:w0 + 2, :, :])
                        xns.append(xn)
                te.stage = "transp"
                for k in range(KB):
                    pt = ps_big.tile([128, CH], BF16, tag="big", name=f"pt_{p}_{c}_{k}")
                    for tb in range(4):
                        te.transpose(
                            pt[:, tb * 128:(tb + 1) * 128],
                            xns[tb][:, k * 128:(k + 1) * 128], ident[:])
                    nc.vector.tensor_copy(xt[k][:], pt[:])

                # 2. qkT projection: 16 m-tiles, accumulate over 8 k-blocks
                qkt = [p_qkt.tile([128, CH], BF16, tag="qkt", name=f"qkt_{p}_{c}_{i}")
                       for i in range(16)]
                te.stage = "qkT"
                for m in range(16):
                    pq = ps_big.tile([128, CH], F32, tag="big", name=f"pq_{p}_{c}_{m}")
                    for k in range(KB):
                        te.matmul(
                            pq[:],
                            lhsT=wqk_t[k][:, m * 128:(m + 1) * 128],
                            rhs=xt[k][:],
                            start=(k == 0), stop=(k == KB - 1))
                    nc.vector.tensor_copy(qkt[m][:], pq[:])

                # 3. v projection, natural [tok, d] layout
                v_t = [p_v.tile([128, D], BF16, tag="v", name=f"v_{p}_{c}_{i}") for i in range(4)]
                te.stage = "v"
                for tb in range(4):
                    for n2 in range(2):
                        pv = ps_big.tile([128, CH], F32, tag="big", name=f"pv_{p}_{c}_{tb}_{n2}")
                        for k in range(KB):
                            te.matmul(
                                pv[:],
                                lhsT=xt[k][:, tb * 128:(tb + 1) * 128],
                                rhs=wv_t[k][:, n2 * 512:(n2 + 1) * 512],
                                start=(k == 0), stop=(k == KB - 1))
                        nc.vector.tensor_copy(
                            v_t[tb][:, n2 * 512:(n2 + 1) * 512], pv[:])

                # 4. attention, batched per head-pair j: all 8 sequences'
                # A^T (and O^T) land in one PSUM bank per PE row-tile
                # (row tiles must not share a bank), 16 dense 64x64
                # matmuls per bank pair, then one copy per bank.
                # paE = head 2j (row tile 0), paO = head 2j+1 (row tile 1);
                # layout: rows (s%2)*64, cols (s//2)*64.
                ot = [p_ot.tile([128, CH], BF16, tag="ot", name=f"ot_{p}_{c}_{i}") for i in range(8)]
                te.stage = "att"
                # Software pipeline: emit A(j+1) between A(j) and O(j) so
                # the PSUM->SBUF copies of A(j) are off the PE critical
                # path.
                ot = [p_ot.tile([128, CH], BF16, tag="ot", name=f"ot_{p}_{c}_{i}") for i in range(8)]

                def emit_A(j):
                    kq = qkt[8 + j]
                    qq = qkt[j]
                    paE = ps_att.tile([128, 256], F32, tag="att", name=f"paE_{p}_{c}_{j}")
                    paO = ps_att.tile([128, 256], F32, tag="att", name=f"paO_{p}_{c}_{j}")
                    for s in range(8):
                        rp = (s % 2) * 64
                        fc = (s // 2) * 64
                        ssl = slice(s * 64, (s + 1) * 64)
                        te.matmul(
                            paE[rp:rp + 64, fc:fc + 64],
                            lhsT=kq[0:64, ssl], rhs=qq[0:64, ssl],
                            start=True, stop=True, tile_position=(0, rp))
                        te.matmul(
                            paO[rp:rp + 64, fc:fc + 64],
                            lhsT=kq[64:128, ssl], rhs=qq[64:128, ssl],
                            start=True, stop=True, tile_position=(64, rp))
                    saE = p_sa.tile([128, 256], BF16, tag="sa", name=f"saE_{p}_{c}_{j}")
                    saO = p_sa.tile([128, 256], BF16, tag="sa", name=f"saO_{p}_{c}_{j}")
                    nc.scalar.copy(saE[:], paE[:])
                    nc.vector.tensor_copy(saO[:], paO[:])
                    return saE, saO

                def emit_O(j, saE, saO):
                    poS0 = ps_att.tile([128, 256], F32, tag="att", name=f"poS0_{p}_{c}_{j}")
                    poS1 = ps_att.tile([128, 256], F32, tag="att", name=f"poS1_{p}_{c}_{j}")
                    h0 = slice((2 * j) * 64, (2 * j + 1) * 64)
                    h1 = slice((2 * j + 1) * 64, (2 * j + 2) * 64)
                    for s in range(8):
                        rv = (s % 2) * 64
                        fc = (s // 2) * 64
                        vv = v_t[s // 2]
                        dst = poS0 if s % 2 == 0 else poS1
                        te.matmul(
                            dst[0:64, fc:fc + 64],
                            lhsT=vv[rv:rv + 64, h0],
                            rhs=saE[rv:rv + 64, fc:fc + 64],
                            start=True, stop=True, tile_position=(rv, 0))
                        te.matmul(
                            dst[64:128, fc:fc + 64],
                            lhsT=vv[rv:rv + 64, h1],
                            rhs=saO[rv:rv + 64, fc:fc + 64],
                            start=True, stop=True, tile_position=(rv, 64))
                    otv = ot[j].rearrange("p (s2 par t) -> p par s2 t", par=2, t=64)
                    po0v = poS0.rearrange("p (s2 t) -> p s2 t", t=64)
                    po1v = poS1.rearrange("p (s2 t) -> p s2 t", t=64)
                    nc.vector.tensor_copy(otv[:, 0], po0v)
                    nc.vector.tensor_copy(otv[:, 1], po1v)

                pend = None
                for j in range(8):
                    sa_pair = emit_A(j)
                    if pend is not None:
                        emit_O(pend[0], pend[1], pend[2])
                    pend = (j, sa_pair[0], sa_pair[1])
                emit_O(pend[0], pend[1], pend[2])

                te.stage = "y"
                for tb in range(4):
                    ysb = p_y.tile([128, D], F32, tag="y", name=f"y_{p}_{c}_{tb}")
                    for n2 in range(2):
                        py = ps_big.tile([128, CH], F32, tag="big", name=f"py_{p}_{c}_{tb}_{n2}")
                        for i in range(KB):
                            k = (i + tb * 2 + n2) % KB
                            te.matmul(
                                py[:],
                                lhsT=ot[k][:, tb * 128:(tb + 1) * 128],
                                rhs=wo_t[k][:, n2 * 512:(n2 + 1) * 512],
                                start=(i == 0), stop=(i == KB - 1))
                        nc.vector.tensor_copy(
                            ysb[:, n2 * 512:(n2 + 1) * 512], py[:])
                    if p == 1:
                        t0 = c * CH + tb * 128
                        nc.gpsimd.dma_start(
                            out[t0:t0 + 128, :], ysb[:],
                            accum_op=mybir.AluOpType.add)
                    else:
                        w0 = c * 8 + tb * 2
                        yeng = nc.sync if tb % 2 == 0 else nc.scalar
                        yeng.dma_start(og[w0:w0 + 2, :, :], ysb[:])
    nc.compile()
    _BUILD_CACHE[key] = nc
    return nc


def _prep_inputs(x, w_qkv0, w_out0, w_qkv1, w_out1):
    bf = ml_dtypes.bfloat16
    xb = np.ascontiguousarray(x.reshape(B, NT, D)).astype(bf)
    common = {}
    for p, (wqkv, wout) in enumerate(((w_qkv0, w_out0), (w_qkv1, w_out1))):
        wqk_s = np.ascontiguousarray(wqkv[:, :2 * D]).copy()
        wqk_s[:, :D] *= SCALE  # fold q scale into weights (2^-5, exact)
        common[f"wqk{p}"] = wqk_s.astype(bf)
        common[f"wv{p}"] = np.ascontiguousarray(wqkv[:, 2 * D:]).astype(bf)
        common[f"wo{p}"] = np.ascontiguousarray(wout).astype(bf)
    return [{"x": xb[b], **common} for b in range(B)]


def kernel(x, w_qkv0, w_out0, w_qkv1, w_out1, trace=False, tmpdir=None):
    nc = build()
    in_maps = _prep_inputs(x, w_qkv0, w_out0, w_qkv1, w_out1)
    res = run_bass_kernel_spmd(nc, in_maps, core_ids=list(range(B)),
                               trace=trace, tmpdir=tmpdir)
    outs = np.stack([res.results[b]["out"] for b in range(B)])
    outs = outs.reshape(B, 64, 64, D)
    kernel.last_result = res
    return outs


# revision 21
# speedup vs baseline: 1.0038x; 1.0038x over previous
"""Axial attention (no softmax) on 8 TRN2 NeuronCores.

Problem: x (8, 64, 64, 1024) fp32; two self-attentions (16 heads, no
softmax, scale d**-0.5) along the H axis (w_qkv0/w_out0) and the W axis
(w_qkv1/w_out1); output is their sum.

Sharding: data-parallel over batch B=8 -> one batch slab per core,
weights replicated. Each core computes both axial passes for its slab;
no collectives.

Per-core kernel structure (all matmuls bf16, fp32 PSUM accumulate):
  tokens t = h*64 + w (h-major), NT = 4096 per slab.
  For each pass (H-axis then W-axis), in chunks of 8 sequences
  (CH = 512 tokens, chunk token order is sequence-major):
    1. DMA natural x tiles [128 tok, 1024], PE-transpose to
       xT [128 d, 512 tok] tiles (8 k-tiles per chunk).
    2. qkT[m] = (Wqk[:, m-block]).T @ xT  -> [128 qk-dim, 512 tok]
       (16 m-tiles, 8 k accumulation steps each; q scaled by 1/32).
    3. v[tb] = x @ Wv -> [128 tok, 1024] natural layout (4 tok-blocks).
    4. Per (head-pair j, seq-pair sp): 4-way 64x64 tile_position packs:
       A^T = kT.T @ qT   (4 matmuls into one PSUM tile)
       O^T = v.T  @ A^T  (4 matmuls into one PSUM tile)
       assembling OT[j] [128 d, 512 tok].
    5. y = OT.T @ Wout -> [128 tok, 512] fp32; pass H writes out
       directly, pass W gpsimd-DMA-accumulates (out = oh + ow).
"""

import numpy as np
import ml_dtypes
from contextlib import ExitStack

from concourse.bass_utils import run_bass_kernel_spmd
from concourse import bacc, mybir, tile
from concourse.masks import make_identity

BF16 = mybir.dt.bfloat16
F32 = mybir.dt.float32

B = 8
D = 1024
NT = 4096          # tokens per core (64*64)
CH = 512           # chunk tokens (8 sequences of 64)
NCHUNK = NT // CH  # 8
KB = D // 128      # 8 contraction blocks
SCALE = 1.0 / 32.0  # 1024 ** -0.5

_BUILD_CACHE = {}
STAGE_MAP = {}


class _TensorProxy:
    """Records which pipeline stage emitted each PE instruction (for
    trace attribution in the perf harness)."""

    def __init__(self, te):
        self._te = te
        self.stage = "?"

    def matmul(self, *a, **kw):
        r = self._te.matmul(*a, **kw)
        STAGE_MAP[r.ins.name] = self.stage
        return r

    def transpose(self, *a, **kw):
        r = self._te.transpose(*a, **kw)
        STAGE_MAP[r.ins.name] = self.stage
        return r


def build(n_chunks=NCHUNK, passes=(0, 1)):
    key = (n_chunks, tuple(passes))
    if key in _BUILD_CACHE:
        return _BUILD_CACHE[key]

    nc = bacc.Bacc("TRN2", target_bir_lowering=False, debug=False)
    x = nc.dram_tensor("x", [NT, D], BF16, kind="ExternalInput")
    wqk = [nc.dram_tensor(f"wqk{p}", [D, 2 * D], BF16, kind="ExternalInput")
           for p in range(2)]
    wv = [nc.dram_tensor(f"wv{p}", [D, D], BF16, kind="ExternalInput")
          for p in range(2)]
    wo = [nc.dram_tensor(f"wo{p}", [D, D], BF16, kind="ExternalInput")
          for p in range(2)]
    out = nc.dram_tensor("out", [NT, D], F32, kind="ExternalOutput")

    xg = x.rearrange("(h w) d -> w h d", w=64)    # pass-H gather view
    og = out.rearrange("(h w) d -> w h d", w=64)  # pass-H scatter view

    with tile.TileContext(nc) as tc, ExitStack() as ctx:
        def pool(name, bufs, space="SBUF"):
            return ctx.enter_context(
                tc.tile_pool(name=name, bufs=bufs, space=space))

        p_id = pool("ident", 1)
        p_wqk = pool("wqk", 16)
        p_wv = pool("wv", 8)
        p_wo = pool("wo", 8)
        p_xn = pool("xn", 8)
        p_xt = pool("xt", 16)
        p_qkt = pool("qkt", 16)
        p_v = pool("v", 8)
        p_sa = pool("sa", 10)
        p_ot = pool("ot", 16)
        p_y = pool("y", 6)
        # PSUM budget: 8 banks total (each tile is padded to one bank).
        # Row-tiled 64x64 matmul packs need the two row tiles' outputs in
        # DIFFERENT banks (concurrent row tiles may not share a bank).
        ps_big = pool("psb", 2, "PSUM")    # [128, 512] f32 qkv/y groups
        ps_att = pool("psatt", 6, "PSUM")  # transpose + A^T/O^T halves

        te = _TensorProxy(nc.tensor)
        ident = p_id.tile([128, 128], BF16, name="ident")
        make_identity(nc, ident)

        # PE warm-up: ~5us of dummy matmuls while the first DMAs land,
        # so the HAM clock gate reaches 8/8 before real work starts.
        te.stage = "warm"
        warm_ps = ps_big.tile([128, 128], F32, tag="big", name="warm_ps")
        for _ in range(40):
            te.matmul(warm_ps[:], lhsT=ident[:], rhs=ident[:],
                      start=True, stop=True)

        for p in passes:
            if p == passes[0]:
                # prefetch chunk-0 x tiles ahead of the weight stream
                pre_xns = []
                for tb in range(4):
                    xn = p_xn.tile([128, D], BF16, tag="xn", name=f"xn_pre_{p}_{tb}")
                    engs = (nc.sync, nc.scalar, nc.gpsimd)
                    eng = engs[tb % 3]
                    if p == 1:
                        eng.dma_start(xn[:], x[tb * 128:(tb + 1) * 128, :])
                    else:
                        eng.dma_start(xn[:], xg[tb * 2:tb * 2 + 2, :, :])
                    pre_xns.append(xn)
            else:
                pre_xns = None
            wqk_t = []
            for k in range(KB):
                t = p_wqk.tile([128, 2 * D], BF16, tag="wqk", name=f"wqk_{p}_{k}")
                nc.sync.dma_start(t[:], wqk[p][k * 128:(k + 1) * 128, :])
                wqk_t.append(t)
            wv_t = []
            for k in range(KB):
                t = p_wv.tile([128, D], BF16, tag="wv", name=f"wv_{p}_{k}")
                nc.scalar.dma_start(t[:], wv[p][k * 128:(k + 1) * 128, :])
                wv_t.append(t)
            wo_t = []
            for k in range(KB):
                t = p_wo.tile([128, D], BF16, tag="wo", name=f"wo_{p}_{k}")
                nc.scalar.dma_start(t[:], wo[p][k * 128:(k + 1) * 128, :])
                wo_t.append(t)

            for c in range(n_chunks):
                # 1. load natural x tiles, PE-transpose into xT k-tiles.
                # All 4 transposes of one k-block go into one [128, 512]
                # PSUM tile (one bank, one copy out).
                xt = [p_xt.tile([128, CH], BF16, tag="xt", name=f"xt_{p}_{c}_{i}") for i in range(KB)]
                if c == 0 and pre_xns is not None:
                    xns = pre_xns
                else:
                    xns = []
                    for tb in range(4):
                        xn = p_xn.tile([128, D], BF16, tag="xn", name=f"xn_{p}_{c}_{tb}")
                        engs = (nc.sync, nc.scalar, nc.gpsimd)
                        eng = engs[(c * 4 + tb) % 3]
                        if p == 1:
                            t0 = c * CH + tb * 128
                            eng.dma_start(xn[:], x[t0:t0 + 128, :])
                        else:
                            w0 = c * 8 + tb * 2
                            eng.dma_start(xn[:], xg[w0:w0 + 2, :, :])
                        xns.append(xn)
                te.stage = "transp"
                for k in range(KB):
                    pt = ps_big.tile([128, CH], BF16, tag="big", name=f"pt_{p}_{c}_{k}")
                    for tb in range(4):
                        te.transpose(
                            pt[:, tb * 128:(tb + 1) * 128],
                            xns[tb][:, k * 128:(k + 1) * 128], ident[:])
                    nc.vector.tensor_copy(xt[k][:], pt[:])

                # 2. qkT projection: 16 m-tiles, accumulate over 8 k-blocks
                qkt = [p_qkt.tile([128, CH], BF16, tag="qkt", name=f"qkt_{p}_{c}_{i}")
                       for i in range(16)]
                te.stage = "qkT"
                for m in range(16):
                    pq = ps_big.tile([128, CH], F32, tag="big", name=f"pq_{p}_{c}_{m}")
                    for k in range(KB):
                        te.matmul(
                            pq[:],
                            lhsT=wqk_t[k][:, m * 128:(m + 1) * 128],
                            rhs=xt[k][:],
                            start=(k == 0), stop=(k == KB - 1))
                    nc.vector.tensor_copy(qkt[m][:], pq[:])

                # 3. v projection, natural [tok, d] layout
                v_t = [p_v.tile([128, D], BF16, tag="v", name=f"v_{p}_{c}_{i}") for i in range(4)]
                te.stage = "v"
                for tb in range(4):
                    for n2 in range(2):
                        pv = ps_big.tile([128, CH], F32, tag="big", name=f"pv_{p}_{c}_{tb}_{n2}")
                        for k in range(KB):
                            te.matmul(
                                pv[:],
                                lhsT=xt[k][:, tb * 128:(tb + 1) * 128],
                                rhs=wv_t[k][:, n2 * 512:(n2 + 1) * 512],
                                start=(k == 0), stop=(k == KB - 1))
                        nc.vector.tensor_copy(
                            v_t[tb][:, n2 * 512:(n2 + 1) * 512], pv[:])

                # 4. attention, batched per head-pair j: all 8 sequences'
                # A^T (and O^T) land in one PSUM bank per PE row-tile
                # (row tiles must not share a bank), 16 dense 64x64
                # matmuls per bank pair, then one copy per bank.
                # paE = head 2j (row tile 0), paO = head 2j+1 (row tile 1);
                # layout: rows (s%2)*64, cols (s//2)*64.
                te.stage = "att"
                # Software pipeline: emit A(j+1), A(j+2) between A(j) and
                # O(j) so the PSUM->SBUF copies of A(j) are fully off the
                # PE critical path.
                ot = [p_ot.tile([128, CH], BF16, tag="ot", name=f"ot_{p}_{c}_{i}") for i in range(8)]

                def emit_A(j):
                    kq = qkt[8 + j]
                    qq = qkt[j]
                    paE = ps_att.tile([128, 256], F32, tag="att", name=f"paE_{p}_{c}_{j}")
                    paO = ps_att.tile([128, 256], F32, tag="att", name=f"paO_{p}_{c}_{j}")
                    for s in range(8):
                        rp = (s % 2) * 64
                        fc = (s // 2) * 64
                        ssl = slice(s * 64, (s + 1) * 64)
                        te.matmul(
                            paE[rp:rp + 64, fc:fc + 64],
                            lhsT=kq[0:64, ssl], rhs=qq[0:64, ssl],
                            start=True, stop=True, tile_position=(0, rp))
                        te.matmul(
                            paO[rp:rp + 64, fc:fc + 64],
                            lhsT=kq[64:128, ssl], rhs=qq[64:128, ssl],
                            start=True, stop=True, tile_position=(64, rp))
                    saE = p_sa.tile([128, 256], BF16, tag="sa", name=f"saE_{p}_{c}_{j}")
                    saO = p_sa.tile([128, 256], BF16, tag="sa", name=f"saO_{p}_{c}_{j}")
                    nc.scalar.copy(saE[:], paE[:])
                    nc.vector.tensor_copy(saO[:], paO[:])
                    return saE, saO

                def emit_O(j, saE, saO):
                    poS0 = ps_att.tile([128, 256], F32, tag="att", name=f"poS0_{p}_{c}_{j}")
                    poS1 = ps_att.tile([128, 256], F32, tag="att", name=f"poS1_{p}_{c}_{j}")
                    h0 = slice((2 * j) * 64, (2 * j + 1) * 64)
                    h1 = slice((2 * j + 1) * 64, (2 * j + 2) * 64)
                    for s in range(8):
                        rv = (s % 2) * 64
                        fc = (s // 2) * 64
                        vv = v_t[s // 2]
                        dst = poS0 if s % 2 == 0 else poS1
                        te.matmul(
                            dst[0:64, fc:fc + 64],
                            lhsT=vv[rv:rv + 64, h0],
                            rhs=saE[rv:rv + 64, fc:fc + 64],
                            start=True, stop=True, tile_position=(rv, 0))
                        te.matmul(
                            dst[64:128, fc:fc + 64],
                            lhsT=vv[rv:rv + 64, h1],
                            rhs=saO[rv:rv + 64, fc:fc + 64],
                            start=True, stop=True, tile_position=(rv, 64))
                    otv = ot[j].rearrange("p (s2 par t) -> p par s2 t", par=2, t=64)
                    po0v = poS0.rearrange("p (s2 t) -> p s2 t", t=64)
                    po1v = poS1.rearrange("p (s2 t) -> p s2 t", t=64)
                    nc.vector.tensor_copy(otv[:, 0], po0v)
                    nc.vector.tensor_copy(otv[:, 1], po1v)

                pend = []
                for j in range(8):
                    sa_pair = emit_A(j)
                    if len(pend) >= 2:
                        oj = pend.pop(0)
                        emit_O(oj[0], oj[1], oj[2])
                    pend.append((j, sa_pair[0], sa_pair[1]))
                for oj in pend:
                    emit_O(oj[0], oj[1], oj[2])

                te.stage = "y"
                for tb in range(4):
                    ysb = p_y.tile([128, D], F32, tag="y", name=f"y_{p}_{c}_{tb}")
                    for n2 in range(2):
                        py = ps_big.tile([128, CH], F32, tag="big", name=f"py_{p}_{c}_{tb}_{n2}")
                        for i in range(KB):
                            k = (i + tb * 2 + n2) % KB
                            te.matmul(
                                py[:],
                                lhsT=ot[k][:, tb * 128:(tb + 1) * 128],
                                rhs=wo_t[k][:, n2 * 512:(n2 + 1) * 512],
                                start=(i == 0), stop=(i == KB - 1))
                        nc.vector.tensor_copy(
                            ysb[:, n2 * 512:(n2 + 1) * 512], py[:])
                    if p == 1:
                        t0 = c * CH + tb * 128
                        nc.gpsimd.dma_start(
                            out[t0:t0 + 128, :], ysb[:],
                            accum_op=mybir.AluOpType.add)
                    else:
                        w0 = c * 8 + tb * 2
                        yeng = nc.sync if tb % 2 == 0 else nc.scalar
                        yeng.dma_start(og[w0:w0 + 2, :, :], ysb[:])
    nc.compile()
    _BUILD_CACHE[key] = nc
    return nc


def _prep_inputs(x, w_qkv0, w_out0, w_qkv1, w_out1):
    bf = ml_dtypes.bfloat16
    xb = np.ascontiguousarray(x.reshape(B, NT, D)).astype(bf)
    common = {}
    for p, (wqkv, wout) in enumerate(((w_qkv0, w_out0), (w_qkv1, w_out1))):
        wqk_s = np.ascontiguousarray(wqkv[:, :2 * D]).copy()
        wqk_s[:, :D] *= SCALE  # fold q scale into weights (2^-5, exact)
        common[f"wqk{p}"] = wqk_s.astype(bf)
        common[f"wv{p}"] = np.ascontiguousarray(wqkv[:, 2 * D:]).astype(bf)
        common[f"wo{p}"] = np.ascontiguousarray(wout).astype(bf)
    return [{"x": xb[b], **common} for b in range(B)]


def kernel(x, w_qkv0, w_out0, w_qkv1, w_out1, trace=False, tmpdir=None):
    nc = build()
    in_maps = _prep_inputs(x, w_qkv0, w_out0, w_qkv1, w_out1)
    res = run_bass_kernel_spmd(nc, in_maps, core_ids=list(range(B)),
                               trace=trace, tmpdir=tmpdir)
    outs = np.stack([res.results[b]["out"] for b in range(B)])
    outs = outs.reshape(B, 64, 64, D)
    kernel.last_result = res
    return outs


# revision 22
# speedup vs baseline: 1.0188x; 1.0150x over previous
"""Axial attention (no softmax) on 8 TRN2 NeuronCores.

Problem: x (8, 64, 64, 1024) fp32; two self-attentions (16 heads, no
softmax, scale d**-0.5) along the H axis (w_qkv0/w_out0) and the W axis
(w_qkv1/w_out1); output is their sum.

Sharding: data-parallel over batch B=8 -> one batch slab per core,
weights replicated. Each core computes both axial passes for its slab;
no collectives.

Per-core kernel structure (all matmuls bf16, fp32 PSUM accumulate):
  tokens t = h*64 + w (h-major), NT = 4096 per slab.
  For each pass (H-axis then W-axis), in chunks of 8 sequences
  (CH = 512 tokens, chunk token order is sequence-major):
    1. DMA natural x tiles [128 tok, 1024], PE-transpose to
       xT [128 d, 512 tok] tiles (8 k-tiles per chunk).
    2. qkT[m] = (Wqk[:, m-block]).T @ xT  -> [128 qk-dim, 512 tok]
       (16 m-tiles, 8 k accumulation steps each; q scaled by 1/32).
    3. v[tb] = x @ Wv -> [128 tok, 1024] natural layout (4 tok-blocks).
    4. Per (head-pair j, seq-pair sp): 4-way 64x64 tile_position packs:
       A^T = kT.T @ qT   (4 matmuls into one PSUM tile)
       O^T = v.T  @ A^T  (4 matmuls into one PSUM tile)
       assembling OT[j] [128 d, 512 tok].
    5. y = OT.T @ Wout -> [128 tok, 512] fp32; pass H writes out
       directly, pass W gpsimd-DMA-accumulates (out = oh + ow).
"""

import numpy as np
import ml_dtypes
from contextlib import ExitStack

from concourse.bass_utils import run_bass_kernel_spmd
from concourse import bacc, mybir, tile
from concourse.masks import make_identity

BF16 = mybir.dt.bfloat16
F32 = mybir.dt.float32

B = 8
D = 1024
NT = 4096          # tokens per core (64*64)
CH = 512           # chunk tokens (8 sequences of 64)
NCHUNK = NT // CH  # 8
KB = D // 128      # 8 contraction blocks
SCALE = 1.0 / 32.0  # 1024 ** -0.5

_BUILD_CACHE = {}
STAGE_MAP = {}


class _TensorProxy:
    """Records which pipeline stage emitted each PE instruction (for
    trace attribution in the perf harness)."""

    def __init__(self, te):
        self._te = te
        self.stage = "?"

    def matmul(self, *a, **kw):
        r = self._te.matmul(*a, **kw)
        STAGE_MAP[r.ins.name] = self.stage
        return r

    def transpose(self, *a, **kw):
        r = self._te.transpose(*a, **kw)
        STAGE_MAP[r.ins.name] = self.stage
        return r


def build(n_chunks=NCHUNK, passes=(0, 1)):
    key = (n_chunks, tuple(passes))
    if key in _BUILD_CACHE:
        return _BUILD_CACHE[key]

    nc = bacc.Bacc("TRN2", target_bir_lowering=False, debug=False)
    x = nc.dram_tensor("x", [NT, D], BF16, kind="ExternalInput")
    wqk = [nc.dram_tensor(f"wqk{p}", [D, 2 * D], BF16, kind="ExternalInput")
           for p in range(2)]
    wv = [nc.dram_tensor(f"wv{p}", [D, D], BF16, kind="ExternalInput")
          for p in range(2)]
    wo = [nc.dram_tensor(f"wo{p}", [D, D], BF16, kind="ExternalInput")
          for p in range(2)]
    out = nc.dram_tensor("out", [NT, D], F32, kind="ExternalOutput")

    xg = x.rearrange("(h w) d -> w h d", w=64)    # pass-H gather view
    og = out.rearrange("(h w) d -> w h d", w=64)  # pass-H scatter view

    with tile.TileContext(nc) as tc, ExitStack() as ctx:
        def pool(name, bufs, space="SBUF"):
            return ctx.enter_context(
                tc.tile_pool(name=name, bufs=bufs, space=space))

        p_id = pool("ident", 1)
        p_wqk = pool("wqk", 16)
        p_wv = pool("wv", 8)
        p_wo = pool("wo", 8)
        p_xn = pool("xn", 8)
        p_xt = pool("xt", 16)
        p_qkt = pool("qkt", 16)
        p_v = pool("v", 8)
        p_sa = pool("sa", 10)
        p_ot = pool("ot", 16)
        p_y = pool("y", 6)
        # PSUM budget: 8 banks total (each tile is padded to one bank).
        # Row-tiled 64x64 matmul packs need the two row tiles' outputs in
        # DIFFERENT banks (concurrent row tiles may not share a bank).
        ps_big = pool("psb", 3, "PSUM")    # [128, 512] f32 qkv/y groups
        ps_att = pool("psatt", 5, "PSUM")  # transpose + A^T/O^T halves

        te = _TensorProxy(nc.tensor)
        ident = p_id.tile([128, 128], BF16, name="ident")
        make_identity(nc, ident)

        # PE warm-up: ~5us of dummy matmuls while the first DMAs land,
        # so the HAM clock gate reaches 8/8 before real work starts.
        te.stage = "warm"
        warm_ps = ps_big.tile([128, 128], F32, tag="big", name="warm_ps")
        for _ in range(40):
            te.matmul(warm_ps[:], lhsT=ident[:], rhs=ident[:],
                      start=True, stop=True)

        for p in passes:
            if p == passes[0]:
                # prefetch chunk-0 x tiles ahead of the weight stream
                pre_xns = []
                for tb in range(4):
                    xn = p_xn.tile([128, D], BF16, tag="xn", name=f"xn_pre_{p}_{tb}")
                    engs = (nc.sync, nc.scalar, nc.gpsimd)
                    eng = engs[tb % 3]
                    if p == 1:
                        eng.dma_start(xn[:], x[tb * 128:(tb + 1) * 128, :])
                    else:
                        eng.dma_start(xn[:], xg[tb * 2:tb * 2 + 2, :, :])
                    pre_xns.append(xn)
            else:
                pre_xns = None
            wqk_t = []
            for k in range(KB):
                t = p_wqk.tile([128, 2 * D], BF16, tag="wqk", name=f"wqk_{p}_{k}")
                nc.sync.dma_start(t[:], wqk[p][k * 128:(k + 1) * 128, :])
                wqk_t.append(t)
            wv_t = []
            for k in range(KB):
                t = p_wv.tile([128, D], BF16, tag="wv", name=f"wv_{p}_{k}")
                nc.scalar.dma_start(t[:], wv[p][k * 128:(k + 1) * 128, :])
                wv_t.append(t)
            wo_t = []
            for k in range(KB):
                t = p_wo.tile([128, D], BF16, tag="wo", name=f"wo_{p}_{k}")
                nc.scalar.dma_start(t[:], wo[p][k * 128:(k + 1) * 128, :])
                wo_t.append(t)

            for c in range(n_chunks):
                # 1. load natural x tiles, PE-transpose into xT k-tiles.
                # All 4 transposes of one k-block go into one [128, 512]
                # PSUM tile (one bank, one copy out).
                xt = [p_xt.tile([128, CH], BF16, tag="xt", name=f"xt_{p}_{c}_{i}") for i in range(KB)]
                if c == 0 and pre_xns is not None:
                    xns = pre_xns
                else:
                    xns = []
                    for tb in range(4):
                        xn = p_xn.tile([128, D], BF16, tag="xn", name=f"xn_{p}_{c}_{tb}")
                        engs = (nc.sync, nc.scalar, nc.gpsimd)
                        eng = engs[(c * 4 + tb) % 3]
                        if p == 1:
                            t0 = c * CH + tb * 128
                            eng.dma_start(xn[:], x[t0:t0 + 128, :])
                        else:
                            w0 = c * 8 + tb * 2
                            eng.dma_start(xn[:], xg[w0:w0 + 2, :, :])
                        xns.append(xn)
                te.stage = "transp"
                for k in range(KB):
                    pt = ps_big.tile([128, CH], BF16, tag="big", name=f"pt_{p}_{c}_{k}")
                    for tb in range(4):
                        te.transpose(
                            pt[:, tb * 128:(tb + 1) * 128],
                            xns[tb][:, k * 128:(k + 1) * 128], ident[:])
                    nc.vector.tensor_copy(xt[k][:], pt[:])

                # 2. qkT projection: 16 m-tiles, accumulate over 8 k-blocks
                qkt = [p_qkt.tile([128, CH], BF16, tag="qkt", name=f"qkt_{p}_{c}_{i}")
                       for i in range(16)]
                te.stage = "qkT"
                for m in range(16):
                    pq = ps_big.tile([128, CH], F32, tag="big", name=f"pq_{p}_{c}_{m}")
                    for k in range(KB):
                        te.matmul(
                            pq[:],
                            lhsT=wqk_t[k][:, m * 128:(m + 1) * 128],
                            rhs=xt[k][:],
                            start=(k == 0), stop=(k == KB - 1))
                    nc.vector.tensor_copy(qkt[m][:], pq[:])

                # 3. v projection, natural [tok, d] layout
                v_t = [p_v.tile([128, D], BF16, tag="v", name=f"v_{p}_{c}_{i}") for i in range(4)]
                te.stage = "v"
                for tb in range(4):
                    for n2 in range(2):
                        pv = ps_big.tile([128, CH], F32, tag="big", name=f"pv_{p}_{c}_{tb}_{n2}")
                        for k in range(KB):
                            te.matmul(
                                pv[:],
                                lhsT=xt[k][:, tb * 128:(tb + 1) * 128],
                                rhs=wv_t[k][:, n2 * 512:(n2 + 1) * 512],
                                start=(k == 0), stop=(k == KB - 1))
                        nc.vector.tensor_copy(
                            v_t[tb][:, n2 * 512:(n2 + 1) * 512], pv[:])

                # 4. attention, batched per head-pair j: all 8 sequences'
                # A^T (and O^T) land in one PSUM bank per PE row-tile
                # (row tiles must not share a bank), 16 dense 64x64
                # matmuls per bank pair, then one copy per bank.
                # paE = head 2j (row tile 0), paO = head 2j+1 (row tile 1);
                # layout: rows (s%2)*64, cols (s//2)*64.
                te.stage = "att"
                # Software pipeline: emit A(j+1), A(j+2) between A(j) and
                # O(j) so the PSUM->SBUF copies of A(j) are fully off the
                # PE critical path.
                ot = [p_ot.tile([128, CH], BF16, tag="ot", name=f"ot_{p}_{c}_{i}") for i in range(8)]

                def emit_A(j):
                    kq = qkt[8 + j]
                    qq = qkt[j]
                    paE = ps_att.tile([128, 256], F32, tag="att", name=f"paE_{p}_{c}_{j}")
                    paO = ps_att.tile([128, 256], F32, tag="att", name=f"paO_{p}_{c}_{j}")
                    for s in range(8):
                        rp = (s % 2) * 64
                        fc = (s // 2) * 64
                        ssl = slice(s * 64, (s + 1) * 64)
                        te.matmul(
                            paE[rp:rp + 64, fc:fc + 64],
                            lhsT=kq[0:64, ssl], rhs=qq[0:64, ssl],
                            start=True, stop=True, tile_position=(0, rp))
                        te.matmul(
                            paO[rp:rp + 64, fc:fc + 64],
                            lhsT=kq[64:128, ssl], rhs=qq[64:128, ssl],
                            start=True, stop=True, tile_position=(64, rp))
                    saE = p_sa.tile([128, 256], BF16, tag="sa", name=f"saE_{p}_{c}_{j}")
                    saO = p_sa.tile([128, 256], BF16, tag="sa", name=f"saO_{p}_{c}_{j}")
                    nc.scalar.copy(saE[:], paE[:])
                    nc.vector.tensor_copy(saO[:], paO[:])
                    return saE, saO

                def emit_O(j, saE, saO):
                    poS0 = ps_att.tile([128, 256], F32, tag="att", name=f"poS0_{p}_{c}_{j}")
                    poS1 = ps_att.tile([128, 256], F32, tag="att", name=f"poS1_{p}_{c}_{j}")
                    h0 = slice((2 * j) * 64, (2 * j + 1) * 64)
                    h1 = slice((2 * j + 1) * 64, (2 * j + 2) * 64)
                    for s in range(8):
                        rv = (s % 2) * 64
                        fc = (s // 2) * 64
                        vv = v_t[s // 2]
                        dst = poS0 if s % 2 == 0 else poS1
                        te.matmul(
                            dst[0:64, fc:fc + 64],
                            lhsT=vv[rv:rv + 64, h0],
                            rhs=saE[rv:rv + 64, fc:fc + 64],
                            start=True, stop=True, tile_position=(rv, 0))
                        te.matmul(
                            dst[64:128, fc:fc + 64],
                            lhsT=vv[rv:rv + 64, h1],
                            rhs=saO[rv:rv + 64, fc:fc + 64],
                            start=True, stop=True, tile_position=(rv, 64))
                    otv = ot[j].rearrange("p (s2 par t) -> p par s2 t", par=2, t=64)
                    po0v = poS0.rearrange("p (s2 t) -> p s2 t", t=64)
                    po1v = poS1.rearrange("p (s2 t) -> p s2 t", t=64)
                    nc.vector.tensor_copy(otv[:, 0], po0v)
                    nc.vector.tensor_copy(otv[:, 1], po1v)

                pend = []
                for j in range(8):
                    sa_pair = emit_A(j)
                    if len(pend) >= 2:
                        oj = pend.pop(0)
                        emit_O(oj[0], oj[1], oj[2])
                    pend.append((j, sa_pair[0], sa_pair[1]))
                for oj in pend:
                    emit_O(oj[0], oj[1], oj[2])

                te.stage = "y"
                for tb in range(4):
                    ysb = p_y.tile([128, D], F32, tag="y", name=f"y_{p}_{c}_{tb}")
                    for n2 in range(2):
                        py = ps_big.tile([128, CH], F32, tag="big", name=f"py_{p}_{c}_{tb}_{n2}")
                        for i in range(KB):
                            k = (i + tb * 2 + n2) % KB
                            te.matmul(
                                py[:],
                                lhsT=ot[k][:, tb * 128:(tb + 1) * 128],
                                rhs=wo_t[k][:, n2 * 512:(n2 + 1) * 512],
                                start=(i == 0), stop=(i == KB - 1))
                        nc.vector.tensor_copy(
                            ysb[:, n2 * 512:(n2 + 1) * 512], py[:])
                    if p == 1:
                        t0 = c * CH + tb * 128
                        nc.gpsimd.dma_start(
                            out[t0:t0 + 128, :], ysb[:],
                            accum_op=mybir.AluOpType.add)
                    else:
                        w0 = c * 8 + tb * 2
                        yeng = nc.sync if tb % 2 == 0 else nc.scalar
                        yeng.dma_start(og[w0:w0 + 2, :, :], ysb[:])
    nc.compile()
    _BUILD_CACHE[key] = nc
    return nc


def _prep_inputs(x, w_qkv0, w_out0, w_qkv1, w_out1):
    bf = ml_dtypes.bfloat16
    xb = np.ascontiguousarray(x.reshape(B, NT, D)).astype(bf)
    common = {}
    for p, (wqkv, wout) in enumerate(((w_qkv0, w_out0), (w_qkv1, w_out1))):
        wqk_s = np.ascontiguousarray(wqkv[:, :2 * D]).copy()
        wqk_s[:, :D] *= SCALE  # fold q scale into weights (2^-5, exact)
        common[f"wqk{p}"] = wqk_s.astype(bf)
        common[f"wv{p}"] = np.ascontiguousarray(wqkv[:, 2 * D:]).astype(bf)
        common[f"wo{p}"] = np.ascontiguousarray(wout).astype(bf)
    return [{"x": xb[b], **common} for b in range(B)]


def kernel(x, w_qkv0, w_out0, w_qkv1, w_out1, trace=False, tmpdir=None):
    nc = build()
    in_maps = _prep_inputs(x, w_qkv0, w_out0, w_qkv1, w_out1)
    res = run_bass_kernel_spmd(nc, in_maps, core_ids=list(range(B)),
                               trace=trace, tmpdir=tmpdir)
    outs = np.stack([res.results[b]["out"] for b in range(B)])
    outs = outs.reshape(B, 64, 64, D)
    kernel.last_result = res
    return outs
